# revision 1
# baseline (speedup 1.0000x reference)
"""AttentionBlock (b=2, c=512, 64x64) on 8 trn2 NeuronCores.

Sharding: core i handles batch i//4, query rows (i%4)*1024..+1024 (of the
4096 flattened h*w positions). Each core receives its batch's full x with
columns rotated so its own query block sits at columns 0:1024, computes
LayerNorm + K + V for all 4096 positions (replicated inside the 4-core
batch group) and Q/attention/projection for its 1024 queries.

Math reformulation (validated against the jax reference):
  - norm_w and the LayerNorm centering are folded into the QKV weights on
    the host: W~ = (W*norm_w) - row_mean(W*norm_w); then
    qkv = rsqrt(var+eps) * (W~ @ x) + b  -- no on-device mean subtraction.
  - rsqrt(C*(var+eps)) is computed on the DVE with the Quake bit-trick +
    one Newton step (no ACT tables), scaled by 16 so x' = 16*r*x sits in
    fp8's sweet spot.
  - all large matmuls run in fp8e4m3 with MatmulPerfMode.DoubleRow (two
    128-deep k-tiles per pass, 2x PE throughput). Scale bookkeeping:
      x8 = fp8(16*r*x);  W*8 = fp8(sqrt(C)*W~)
      k8 = fp8(kp/16 + bk)              (std ~1)
      q8 = fp8(qp * 32/(16*sqrt(C)) + 32*bq)   = 32*logit-ready q
      v8 = fp8(vp/16)                   (std ~1, V bias folded into proj b)
      st = k8^T q8 = 32*logits;  p8 = fp8(exp(st/32 - 1.5))
      sumexp via ones^T p8 matmul (consistent with p8 => exact softmax)
      avn8 = fp8(av * 32/sumexp);  out = pop/(32*sqrt(C)) + bp + x
  - x is DMA'd once into SBUF and reused for stats, x', and the residual.
  - evictions ride GpSimd (Pool); ACT does exclusively Exp (one table set).
"""
import sys

if "/opt/trn_rl_repo" not in sys.path:
    sys.path.insert(0, "/opt/trn_rl_repo")

import numpy as np

C = 512          # channels
N = 4096         # h*w positions
NQ = 1024        # queries per core
PC = 4           # c chunks of 128
NKC = 32         # key chunks of 128
NCH = 16         # x column chunks of 256
CH = 256         # x chunk width
EPS = 1e-5
SQC = 22.627416997969522   # sqrt(512)
X8S = 16.0                 # x'8 = 16 * r * x
Q8S = 32.0                 # q8 = 32 * logit-ready q
EXPB = -1.5                # exp(logits + EXPB), cancels in softmax
MAGIC = 0x5F3759DF         # Quake rsqrt seed

_cached_nc = None


def _build_nc():
    import concourse.bass as bass
    import concourse.tile as tile
    from concourse import bacc, mybir
    from concourse.masks import make_identity

    f32 = mybir.dt.float32
    f32r = mybir.dt.float32r
    i32 = mybir.dt.int32
    f16 = mybir.dt.float16
    f8 = mybir.dt.float8e4
    AF = mybir.ActivationFunctionType
    ALU = mybir.AluOpType
    DR = mybir.MatmulPerfMode.DoubleRow

    nc = bacc.Bacc(None, target_bir_lowering=False)

    xd = nc.declare_dram_parameter("x", [NCH, 128, PC, CH], f32r, isOutput=False)
    wqd = nc.declare_dram_parameter("wq", [128, PC, C], f8, isOutput=False)
    wkd = nc.declare_dram_parameter("wk", [128, PC, C], f8, isOutput=False)
    wvd = nc.declare_dram_parameter("wv", [128, PC, C], f8, isOutput=False)
    wpd = nc.declare_dram_parameter("wp", [128, PC, C], f8, isOutput=False)
    bqd = nc.declare_dram_parameter("bq", [128, PC], f32, isOutput=False)
    bpd = nc.declare_dram_parameter("bp", [128, PC], f32, isOutput=False)
    outd = nc.declare_dram_parameter("out", [C, NQ], f32, isOutput=True)

    outr = outd.rearrange("(a p) n -> p a n", p=128)   # [128, 4, NQ]

    with tile.TileContext(nc) as tc:
        from contextlib import ExitStack

        with ExitStack() as ctx:
            consts = ctx.enter_context(tc.tile_pool(name="consts", bufs=1))
            xpool = ctx.enter_context(tc.tile_pool(name="xpool", bufs=1))
            kvq = ctx.enter_context(tc.tile_pool(name="kvq", bufs=1))
            dramp = ctx.enter_context(
                tc.tile_pool(name="dramp", bufs=1, space="DRAM")
            )
            # PSUM: exactly 8 banks
            psA = ctx.enter_context(
                tc.tile_pool(name="psA", bufs=1, space=bass.MemorySpace.PSUM)
            )
            psB = ctx.enter_context(
                tc.tile_pool(name="psB", bufs=1, space=bass.MemorySpace.PSUM)
            )
            psC = ctx.enter_context(
                tc.tile_pool(name="psC", bufs=2, space=bass.MemorySpace.PSUM)
            )
            psD = ctx.enter_context(
                tc.tile_pool(name="psD", bufs=4, space=bass.MemorySpace.PSUM)
            )
            stage = ctx.enter_context(tc.tile_pool(name="stage", bufs=1))
            x2p = ctx.enter_context(tc.tile_pool(name="x2p", bufs=2))
            rrp = ctx.enter_context(tc.tile_pool(name="rrp", bufs=2))
            xqp = ctx.enter_context(tc.tile_pool(name="xqp", bufs=2))
            ptp = ctx.enter_context(tc.tile_pool(name="ptp", bufs=3))
            avn_pool = ctx.enter_context(tc.tile_pool(name="avn", bufs=4))
            avt_pool = ctx.enter_context(tc.tile_pool(name="avt", bufs=2))
            out_pool = ctx.enter_context(tc.tile_pool(name="outp", bufs=1))
            small = ctx.enter_context(tc.tile_pool(name="small", bufs=2))

            ident16 = consts.tile([128, 128], f16)
            make_identity(nc, ident16)
            ident1 = consts.tile([1, 1], f32)
            nc.vector.memset(ident1, 1.0)
            ones_col = consts.tile([128, 1], f32r)
            nc.vector.memset(ones_col.bitcast(f32), 1.0)
            ones2 = consts.tile([128, 2, 128], f8)
            nc.vector.memset(ones2, 1.0)
            magict = consts.tile([4, CH], i32)
            nc.vector.memset(magict, MAGIC)
            expb = consts.tile([128, 1], f32)
            nc.vector.memset(expb, EXPB)

            bq_sb = consts.tile([128, PC], f32)
            bp_sb = consts.tile([128, PC], f32)
            wq_sb = consts.tile([128, PC, C], f8)
            wk_sb = consts.tile([128, PC, C], f8)
            wv_sb = consts.tile([128, PC, C], f8)
            wp_sb = consts.tile([128, PC, C], f8)

            x_sb = xpool.tile([128, NCH, PC, CH], f32r)
            k_all = kvq.tile([128, PC, N], f8)     # (c, n) layout
            v_all = kvq.tile([128, NKC, C], f8)    # (n, c) layout
            q_all = kvq.tile([128, PC, NQ], f8)    # (c, nq) layout

            r_dram = dramp.tile([1, N], f32)

            dmaeng = [nc.sync, nc.scalar, nc.gpsimd]

            # ---- phase 1 helpers ----
            def stats_chunk(j, stg_row, a):
                """Column sums of x and x^2 for chunk j -> stg_row block a."""
                xv = x_sb[:, j, :, :]
                xsq = x2p.tile([128, PC, CH], f32r, name="xsq")
                nc.gpsimd.tensor_mul(xsq, xv, xv)
                ps = psA.tile([128, 512], f32, tag="ps", name="ps")
                for ci in range(PC):
                    nc.tensor.matmul(
                        ps[0:1, 0:CH], ones_col, xv[:, ci, :],
                        start=(ci == 0), stop=(ci == PC - 1),
                    )
                for ci in range(PC):
                    nc.tensor.matmul(
                        ps[0:1, CH:2 * CH], ones_col, xsq[:, ci, :],
                        start=(ci == 0), stop=(ci == PC - 1),
                    )
                nc.scalar.activation(
                    stg_row[0:1, a * 512:(a + 1) * 512], ps[0:1, :], AF.Copy
                )

            def rchain(qb, stg_row):
                """rt = 16*rsqrt(C*var + C*eps) for the 2 pairs of batch qb,
                written to r_dram. Quake rsqrt + 1 Newton step, DVE only."""
                stg = stage.tile([4, 512], f32, name="stg")
                nc.sync.dma_start(out=stg, in_=stg_row)
                u2 = stage.tile([4, CH], f32, name="u2")
                nc.vector.tensor_mul(u2, stg[:, 0:CH], stg[:, 0:CH])
                z = stage.tile([4, CH], f32, name="z")
                nc.vector.scalar_tensor_tensor(
                    out=z, in0=u2, scalar=-1.0 / C, in1=stg[:, CH:2 * CH],
                    op0=ALU.mult, op1=ALU.add,
                )
                nc.vector.tensor_scalar_add(z, z, C * EPS)
                r0i = stage.tile([4, CH], i32, name="r0i")
                nc.vector.tensor_scalar(
                    out=r0i, in0=z.bitcast(i32), scalar1=1, scalar2=None,
                    op0=ALU.logical_shift_right,
                )
                nc.vector.tensor_sub(r0i, magict, r0i)
                r0 = r0i.bitcast(f32)
                a2 = stage.tile([4, CH], f32, name="a2")
                nc.vector.tensor_mul(a2, r0, r0)
                nc.vector.tensor_mul(a2, a2, z)
                nc.vector.tensor_scalar(
                    out=a2, in0=a2, scalar1=-0.5 * X8S, scalar2=1.5 * X8S,
                    op0=ALU.mult, op1=ALU.add,
                )
                rt = stage.tile([4, CH], f32, name="rt")
                nc.vector.tensor_mul(rt, r0, a2)
                nc.scalar.dma_start(
                    out=r_dram[0:1, qb * 1024:(qb + 1) * 1024], in_=rt
                )

            def kvq_pair(j2):
                """x'8 prep + K/V(/Q) fp8 DoubleRow matmuls for column pair j2."""
                rr = rrp.tile([128, 2 * CH], f32, name="rr")
                dmaeng[j2 % 2].dma_start(
                    out=rr,
                    in_=r_dram[0:1, j2 * 2 * CH:(j2 + 1) * 2 * CH]
                    .to_broadcast([128, 2 * CH]),
                )
                xq = xqp.tile([128, PC, 2 * CH], f8, name="xq")
                for h in range(2):
                    eng = nc.vector if h == 0 else nc.gpsimd
                    eng.tensor_mul(
                        xq[:, :, h * CH:(h + 1) * CH],
                        x_sb[:, 2 * j2 + h, :, :],
                        rr[:, h * CH:(h + 1) * CH]
                        .unsqueeze(1).broadcast_to([128, PC, CH]),
                    )
                for co in range(PC):
                    kp = psB.tile([128, 512], f32, tag="kvq", name="kp")
                    for i2 in range(2):
                        nc.tensor.matmul(
                            kp,
                            wk_sb[:, 2 * i2:2 * i2 + 2, co * 128:(co + 1) * 128],
                            xq[:, 2 * i2:2 * i2 + 2, :],
                            start=(i2 == 0), stop=(i2 == 1), perf_mode=DR,
                        )
                    if co < 2:
                        nc.vector.tensor_scalar_mul(
                            k_all[:, co, j2 * 512:(j2 + 1) * 512], kp,
                            1.0 / X8S,
                        )
                    else:
                        nc.scalar.activation(
                            k_all[:, co, j2 * 512:(j2 + 1) * 512], kp,
                            AF.Copy, scale=1.0 / X8S,
                        )
                for s4 in range(4):
                    vp = psB.tile([128, C], f32, tag="kvq", name="vp")
                    for i2 in range(2):
                        nc.tensor.matmul(
                            vp,
                            xq[:, 2 * i2:2 * i2 + 2, s4 * 128:(s4 + 1) * 128],
                            wv_sb[:, 2 * i2:2 * i2 + 2, :],
                            start=(i2 == 0), stop=(i2 == 1), perf_mode=DR,
                        )
                    nc.scalar.activation(
                        v_all[:, 4 * j2 + s4, :], vp, AF.Copy, scale=1.0 / X8S
                    )
                if j2 < 2:
                    for co in range(PC):
                        qp = psB.tile([128, 512], f32, tag="kvq", name="qp")
                        for i2 in range(2):
                            nc.tensor.matmul(
                                qp,
                                wq_sb[:, 2 * i2:2 * i2 + 2, co * 128:(co + 1) * 128],
                                xq[:, 2 * i2:2 * i2 + 2, :],
                                start=(i2 == 0), stop=(i2 == 1), perf_mode=DR,
                            )
                        nc.vector.tensor_scalar(
                            out=q_all[:, co, j2 * 512:(j2 + 1) * 512], in0=qp,
                            scalar1=Q8S / (X8S * SQC), scalar2=bq_sb[:, co:co + 1],
                            op0=ALU.mult, op1=ALU.add,
                        )

            # ---- attention helpers ----
            grp = [{} for _ in range(2)]

            def att_sts(g, pr):
                """Scores + exp for jk pair pr of group g -> pt2 tile."""
                q0 = g * 512
                pt2 = ptp.tile([128, 2, 512], f8, tag="pt", name="pt2")
                for u in range(2):
                    jk = 2 * pr + u
                    st = psC.tile([128, 512], f32, tag="st", name="st")
                    for i2 in range(2):
                        nc.tensor.matmul(
                            st,
                            k_all[:, 2 * i2:2 * i2 + 2, jk * 128:(jk + 1) * 128],
                            q_all[:, 2 * i2:2 * i2 + 2, q0:q0 + 512],
                            start=(i2 == 0), stop=(i2 == 1), perf_mode=DR,
                        )
                    nc.scalar.activation(
                        pt2[:, u, :], st, AF.Exp, scale=1.0 / Q8S, bias=expb
                    )
                return pt2

            def att_av(g, pr, pt2):
                """exp(s)^T V accumulation + sumexp for jk pair pr."""
                st8 = grp[g]
                for s in range(4):
                    nc.tensor.matmul(
                        st8["avps"][s],
                        pt2[:, :, s * 128:(s + 1) * 128],
                        v_all[:, 2 * pr:2 * pr + 2, :],
                        start=(pr == 0), stop=(pr == 15), perf_mode=DR,
                    )
                sp = psC.tile([128, 512], f32, tag="st", name="sp")
                nc.tensor.matmul(sp, ones2, pt2, start=True, stop=True,
                                 perf_mode=DR)
                if pr == 0:
                    nc.vector.tensor_copy(st8["sepacc"], sp[0:1, :])
                else:
                    nc.vector.tensor_add(
                        st8["sepacc"], st8["sepacc"], sp[0:1, :]
                    )

            def att_emit(g, n):
                """Emit up to n attention pairs for group g (av lags one
                pair behind scores/exp). n=-1 flushes the remainder."""
                st8 = grp[g]
                if "pr" not in st8:
                    st8["pr"] = 0
                    st8["prev"] = None
                    st8["avps"] = [
                        psD.tile([128, C], f32, tag="av", name=f"avp{g}{s}")
                        for s in range(4)
                    ]
                    st8["sepacc"] = small.tile(
                        [1, 512], f32, tag="sepacc", name=f"sepacc{g}"
                    )
                while n != 0 and st8["pr"] < 16:
                    pr = st8["pr"]
                    pt2 = att_sts(g, pr)
                    if st8["prev"] is not None:
                        att_av(g, pr - 1, st8["prev"])
                    st8["prev"] = pt2
                    st8["pr"] = pr + 1
                    n -= 1
                if n != 0 and st8["pr"] == 16 and st8["prev"] is not None:
                    att_av(g, 15, st8["prev"])
                    st8["prev"] = None

            def evict_group(g):
                q0 = g * 512
                st8 = grp[g]
                rc_ps = psA.tile([128, 4], f32, tag="ps", name="rcps")
                for s in range(4):
                    nc.tensor.transpose(
                        rc_ps[:, s:s + 1],
                        st8["sepacc"][0:1, s * 128:(s + 1) * 128],
                        ident1,
                    )
                rc_sb = small.tile([128, 4], f32, name="rcsb")
                nc.vector.reciprocal(rc_sb, rc_ps)

                avns = []
                for s in range(4):
                    avn = avn_pool.tile([128, C], f16, tag="avn", name=f"avn{s}")
                    nc.vector.tensor_scalar(
                        out=avn, in0=st8["avps"][s],
                        scalar1=rc_sb[:, s:s + 1], scalar2=Q8S,
                        op0=ALU.mult, op1=ALU.mult,
                    )
                    avns.append(avn)
                avt = avt_pool.tile([128, PC, 512], f8, name="avt")
                for ci in range(PC):
                    tps = psA.tile([128, 512], f16, tag="ps", name="tps")
                    for s in range(4):
                        nc.tensor.transpose(
                            tps[:, s * 128:(s + 1) * 128],
                            avns[s][:, ci * 128:(ci + 1) * 128],
                            ident16,
                        )
                    nc.scalar.activation(avt[:, ci, :], tps, AF.Copy)

                out_t = out_pool.tile([128, PC, 512], f32, name="outt")
                for co in range(PC):
                    pop = psB.tile([128, 512], f32, tag="kvq", name="pop")
                    for i2 in range(2):
                        nc.tensor.matmul(
                            pop,
                            wp_sb[:, 2 * i2:2 * i2 + 2, co * 128:(co + 1) * 128],
                            avt[:, 2 * i2:2 * i2 + 2, :],
                            start=(i2 == 0), stop=(i2 == 1), perf_mode=DR,
                        )
                    nc.vector.tensor_scalar(
                        out=out_t[:, co, :], in0=pop,
                        scalar1=1.0 / (Q8S * SQC), scalar2=bp_sb[:, co:co + 1],
                        op0=ALU.mult, op1=ALU.add,
                    )
                    for h in range(2):
                        eng = nc.gpsimd if h == 0 else nc.vector
                        eng.tensor_add(
                            out_t[:, co, h * CH:(h + 1) * CH],
                            out_t[:, co, h * CH:(h + 1) * CH],
                            x_sb[:, 2 * g + h, co, :],
                        )
                        (nc.sync if h == 0 else nc.scalar).dma_start(
                            out=outr[:, co, q0 + h * CH:q0 + (h + 1) * CH],
                            in_=out_t[:, co, h * CH:(h + 1) * CH],
                        )

            # ---- emission: phase 1 (kvq lags stats by one batch), then
            # attention group 0, eviction, group 1, eviction ----
            for qb in range(4):
                if qb == 0:
                    for j in range(8):
                        dmaeng[j % 2].dma_start(out=x_sb[:, j], in_=xd[j])
                    nc.scalar.dma_start(out=bq_sb, in_=bqd[:])
                    nc.sync.dma_start(out=bp_sb, in_=bpd[:])
                    nc.sync.dma_start(out=wk_sb, in_=wkd[:])
                    nc.scalar.dma_start(out=wv_sb, in_=wvd[:])
                    nc.scalar.dma_start(out=wq_sb, in_=wqd[:])
                    nc.sync.dma_start(out=wp_sb, in_=wpd[:])
                else:
                    for cc in range(4):
                        j = 4 * (qb + 1) + cc
                        if j < NCH:
                            dmaeng[j % 2].dma_start(out=x_sb[:, j], in_=xd[j])
                stg_row = stage.tile([1, 2048], f32, name="sr")
                for cc in range(4):
                    stats_chunk(4 * qb + cc, stg_row, cc)
                rchain(qb, stg_row)
                if qb >= 1:
                    kvq_pair(2 * (qb - 1))
                    kvq_pair(2 * (qb - 1) + 1)
                if qb >= 2:
                    att_emit(0, 3)
            kvq_pair(6)
            kvq_pair(7)
            att_emit(0, -1)
            evict_group(0)
            att_emit(1, -1)
            evict_group(1)

    nc.compile()
    return nc


def _get_nc():
    global _cached_nc
    if _cached_nc is None:
        _cached_nc = _build_nc()
    return _cached_nc


def kernel(x, norm_w, w_qkv, b_qkv, w_proj, b_proj):
    import ml_dtypes

    f8np = ml_dtypes.float8_e4m3

    x = np.asarray(x, dtype=np.float32)
    norm_w = np.asarray(norm_w, dtype=np.float32)
    w_qkv = np.asarray(w_qkv, dtype=np.float32)
    b_qkv = np.asarray(b_qkv, dtype=np.float32)
    w_proj = np.asarray(w_proj, dtype=np.float32)
    b_proj = np.asarray(b_proj, dtype=np.float32)

    B = x.shape[0]

    # fold norm_w + LN centering into the QKV weights; sqrt(C) into all
    Wq = w_qkv[0:C] * norm_w[None, :]
    Wk = w_qkv[C:2 * C] * norm_w[None, :]
    Wv = w_qkv[2 * C:3 * C] * norm_w[None, :]

    def wtile(wt):  # [cin, cout] -> [128, PC, cout]
        return np.ascontiguousarray(
            wt.reshape(PC, 128, C).transpose(1, 0, 2).astype(f8np)
        )

    Wqt = wtile(((Wq - Wq.mean(1, keepdims=True)) * SQC).T)
    Wkt = wtile(((Wk - Wk.mean(1, keepdims=True)) * SQC).T)
    Wvt = wtile(((Wv - Wv.mean(1, keepdims=True)) * SQC).T)
    Wpt = wtile(w_proj.T * SQC)

    def cols(b):  # [C] -> [128, 4] chunk-column layout
        return np.ascontiguousarray(b.reshape(PC, 128).T)

    bq = cols(b_qkv[0:C] * (Q8S / SQC))
    bv = b_qkv[2 * C:3 * C]
    bpt = cols(b_proj + w_proj @ bv)

    in_maps = []
    for core in range(8):
        bi, qi = core // 4, core % 4
        xl = np.roll(x[bi].reshape(C, N), -qi * NQ, axis=1)
        # pre-tile to the on-chip layout: [chunk, partition, c-chunk, col]
        xl = np.ascontiguousarray(
            xl.reshape(PC, 128, NCH, CH).transpose(2, 1, 0, 3)
        )
        in_maps.append({
            "x": xl, "wq": Wqt, "wk": Wkt, "wv": Wvt, "wp": Wpt,
            "bq": bq, "bp": bpt,
        })

    from concourse.bass_utils import run_bass_kernel_spmd

    nc = _get_nc()
    res = run_bass_kernel_spmd(nc, in_maps, core_ids=list(range(8)))

    out = np.empty((B, C, N), dtype=np.float32)
    for core in range(8):
        bi, qi = core // 4, core % 4
        out[bi][:, qi * NQ:(qi + 1) * NQ] = res.results[core]["out"]
    return out.reshape(x.shape)

